# revision 8
# baseline (speedup 1.0000x reference)
"""Trainium2 Bass kernel for nn_CrossSemanticAttentionModule0 (cross-modal attention).

Sharding: 8 cores = (batch b in {0,1}) x (query/pixel slab s in {0..3}; 16 H-rows
= 1024 pixels each). Each core computes conv+BN+PReLU for its slab (with halo),
q/k/v projections, AllGathers K (fp16, with a folded-in ones row for the
softmax-max bias) and V^T (bf16) across its 4-core batch group, then computes
both cross-attentions for its query rows over the full key axis and the
up-projections + residuals for its output slab.

Precision: conv -> q/k -> S-logit chain runs in fp16 inputs with fp32 PSUM
accumulation (logit error ~0.1 of +-200-range logits); exp/P/V/O run in bf16
(needed for the subsampled-max headroom: row-max is estimated from every 4th
key, max observed slack ~65 logits, bf16 holds e^65 easily). The flash loop is
software-pipelined: S(t+1) is issued to the PE before O(t) so the exp on the
scalar engine never stalls the tensor engine.
"""

import numpy as np
import functools

import concourse.bass as bass
import concourse.mybir as mybir
import concourse.tile as tile
import concourse.bacc as bacc
from concourse.bass_utils import run_bass_kernel_spmd

B, CIN, H, W = 2, 512, 64, 64
CD, CQ = 256, 32
N = H * W                 # 4096 pixels
SLAB_ROWS = 16            # H rows per core
SLAB = SLAB_ROWS * W      # 1024 pixels per core
HR = SLAB_ROWS + 2        # halo rows
WP = W + 2                # padded width
N_CORES = 8
MODS = ("rgb", "dsm")
F32 = mybir.dt.float32
F16 = mybir.dt.float16
BF16 = mybir.dt.bfloat16
AF = mybir.ActivationFunctionType
RG = [[0, 1, 2, 3], [4, 5, 6, 7]]


def _build():
    nc = bacc.Bacc("TRN2", target_bir_lowering=False, debug=False,
                   num_devices=N_CORES)

    D = {}
    def din(name, shape, dt):
        D[name] = nc.dram_tensor(name, shape, dt, kind="ExternalInput").ap()
    for m in MODS:
        din(f"xs_{m}", [128, 4, HR, WP], F16)
        din(f"cw_{m}", [9, 4, 128, CD], F16)
        din(f"bna_{m}", [128, 2], F32)
        din(f"bnb_{m}", [128, 2], F32)
        din(f"alpha_{m}", [128, 1], F32)
        din(f"gamma_{m}", [1, 1], F32)
        din(f"qkw_{m}", [2, 128, 64], F16)
        din(f"qkb_{m}", [64, 1], F32)
        din(f"vw_{m}", [2, 128, CD], F16)
        din(f"upw_{m}", [2, 128, CIN], F16)
        din(f"upb_{m}", [128, 4], F32)
        din(f"gvb_{m}", [128, 2], F32)
    din("negI", [128, 128], F32)
    OUT = {m: nc.dram_tensor(f"out_{m}", [CIN, SLAB], F32,
                             kind="ExternalOutput").ap() for m in MODS}

    with tile.TileContext(nc) as tc:
        with (
            tc.tile_pool(name="const", bufs=1) as cpool,
            tc.tile_pool(name="cw", bufs=3) as cwpool,
            tc.tile_pool(name="big", bufs=1) as bpool,
            tc.tile_pool(name="pair", bufs=2) as prpool,
            tc.tile_pool(name="pt", bufs=4) as ptpool,
            tc.tile_pool(name="eps", bufs=2) as epool,
            tc.tile_pool(name="ps", bufs=8, space="PSUM") as pp,
            tc.tile_pool(name="dram", bufs=1, space="DRAM") as dpool,
        ):
            # ---- input slabs first (conv is blocked on these) ----
            sb = {}
            for m in MODS:
                t = cpool.tile([128, 4, HR, WP], F16, tag=f"xs_{m}",
                               name=f"xs_{m}")
                nc.sync.dma_start(t[:, 0:2], D[f"xs_{m}"][:, 0:2])
                nc.sync.dma_start(t[:, 2:4], D[f"xs_{m}"][:, 2:4])
                sb[f"xs_{m}"] = t
            # ---- small weights/constants ----
            for m in MODS:
                for nm, shp, dt in (
                    (f"bna_{m}", [128, 2], F32),
                    (f"bnb_{m}", [128, 2], F32),
                    (f"alpha_{m}", [128, 1], F32),
                    (f"gamma_{m}", [1, 1], F32),
                    (f"qkw_{m}", [128, 2, 64], F16),
                    (f"qkb_{m}", [64, 1], F32),
                    (f"vw_{m}", [128, 2, CD], F16),
                    (f"upw_{m}", [128, 2, CIN], F16),
                    (f"upb_{m}", [128, 4], F32),
                    (f"gvb_{m}", [128, 2], F32),
                ):
                    t = cpool.tile(shp, dt, tag=nm, name=nm)
                    src = D[nm]
                    if nm.startswith(("qkw", "vw", "upw")):
                        src = src.rearrange("k p f -> p k f", p=128)
                    nc.sync.dma_start(t[:], src)
                    sb[nm] = t
            negI = cpool.tile([128, 128], F32, tag="negI")
            nc.sync.dma_start(negI[:], D["negI"])
            ones_b = cpool.tile([128, 1], BF16, tag="ones_b")
            nc.vector.memset(ones_b[:], 1.0)
            onesc_b = cpool.tile([1, 128], BF16, tag="onesc_b")
            nc.vector.memset(onesc_b[:], 1.0)
            onesK = cpool.tile([1, SLAB], F16, tag="onesK")
            nc.vector.memset(onesK[:], 1.0)

            # DRAM bounce buffers for collectives (K fp16 + ones row, V bf16)
            kb_in, kb_out, vb_in, vb_out = {}, {}, {}, {}
            for m in MODS:
                kb_in[m] = dpool.tile([CQ + 1, SLAB], F16, tag=f"kbi_{m}", name=f"kbi_{m}")
                kb_out[m] = dpool.tile([4, CQ + 1, SLAB], F16, tag=f"kbo_{m}", name=f"kbo_{m}")
                vb_in[m] = dpool.tile([SLAB, CD], BF16, tag=f"vbi_{m}", name=f"vbi_{m}")
                vb_out[m] = dpool.tile([4, SLAB, CD], BF16, tag=f"vbo_{m}", name=f"vbo_{m}")
                # ones row rides along with K through the AllGather
                nc.sync.dma_start(kb_in[m][CQ:CQ + 1, :], onesK[:])

            conv_sb, convb_sb, qk_sb, vt_sb = {}, {}, {}, {}

            # ---- per-modality: conv -> bn+prelu -> q/k/v projections ----
            for m in MODS:
                xs = sb[f"xs_{m}"]
                conv_sb[m] = bpool.tile([128, 2, SLAB], F16, tag=f"conv_{m}", name=f"conv_{m}")
                convb_sb[m] = bpool.tile([128, 2, SLAB], F32, tag=f"convb_{m}", name=f"convb_{m}")
                qk_sb[m] = bpool.tile([64, SLAB], F16, tag=f"qk_{m}", name=f"qk_{m}")
                vt_sb[m] = bpool.tile([128, 8, CD], BF16, tag=f"vt_{m}", name=f"vt_{m}")

                pcv = [[None, None], [None, None]]
                for mc in range(2):
                    for n2 in range(2):
                        pcv[mc][n2] = pp.tile([128, 512], F32, tag="ps", name=f"pcv_{mc}_{n2}")
                for tap in range(9):
                    dy, dx = tap // 3, tap % 3
                    cwt = cwpool.tile([128, 4, CD], F16, tag="cwt")
                    nc.sync.dma_start(
                        cwt[:], D[f"cw_{m}"][tap].rearrange("k p f -> p k f", p=128))
                    for kc in range(4):
                        for mc in range(2):
                            for n2 in range(2):
                                nc.tensor.matmul(
                                    pcv[mc][n2][:],
                                    cwt[:, kc, 128 * mc:128 * mc + 128],
                                    xs[:, kc, dy + 8 * n2: dy + 8 * n2 + 8,
                                       dx:dx + W],
                                    start=(tap == 0 and kc == 0),
                                    stop=(tap == 8 and kc == 3),
                                )
                for mc in range(2):
                    for n2 in range(2):
                        nc.scalar.activation(
                            conv_sb[m][:, mc, 512 * n2:512 * n2 + 512],
                            pcv[mc][n2][:], AF.Prelu,
                            bias=sb[f"bnb_{m}"][:, mc:mc + 1],
                            scale=sb[f"bna_{m}"][:, mc:mc + 1],
                            alpha=sb[f"alpha_{m}"][:, 0:1],
                        )
                # conv + gamma*v_b (residual-with-v-bias, exact through softmax)
                for mc in range(2):
                    nc.scalar.activation(
                        convb_sb[m][:, mc, :], conv_sb[m][:, mc, :],
                        AF.Identity, bias=sb[f"gvb_{m}"][:, mc:mc + 1])

                # q/k projections (64 = [q;k] channels), then ship K early
                for n2 in range(2):
                    ps = pp.tile([128, 512], F32, tag="ps")
                    for kc in range(2):
                        nc.tensor.matmul(
                            ps[0:64, :], sb[f"qkw_{m}"][:, kc, :],
                            conv_sb[m][:, kc, 512 * n2:512 * n2 + 512],
                            start=(kc == 0), stop=(kc == 1))
                    nc.scalar.activation(
                        qk_sb[m][0:64, 512 * n2:512 * n2 + 512], ps[0:64, :],
                        AF.Identity, bias=sb[f"qkb_{m}"][:, 0:1])
                nc.sync.dma_start(kb_in[m][0:CQ, :], qk_sb[m][32:64, :])
                nc.gpsimd.collective_compute(
                    "AllGather", mybir.AluOpType.bypass, replica_groups=RG,
                    ins=[kb_in[m].opt()], outs=[kb_out[m].opt()])

                # V^T projection ([pix, c] layout, bf16; v bias handled via gvb)
                for pc in range(8):
                    ps = pp.tile([128, 512], F32, tag="ps")
                    for kc in range(2):
                        nc.tensor.matmul(
                            ps[:, 0:CD],
                            conv_sb[m][:, kc, 128 * pc:128 * pc + 128],
                            sb[f"vw_{m}"][:, kc, :],
                            start=(kc == 0), stop=(kc == 1))
                    nc.scalar.activation(vt_sb[m][:, pc, :], ps[:, 0:CD],
                                         AF.Copy)
                nc.sync.dma_start(
                    vb_in[m].rearrange("(pc p) c -> p pc c", p=128), vt_sb[m][:])
                nc.gpsimd.collective_compute(
                    "AllGather", mybir.AluOpType.bypass, replica_groups=RG,
                    ins=[vb_in[m].opt()], outs=[vb_out[m].opt()])

            # ---- attention pairs: (query mod, key/value mod) ----
            for qm, km in (("dsm", "rgb"), ("rgb", "dsm")):
                # K panel: rows 0:32 = K channels, row 32 = ones (bias row)
                KS = prpool.tile([CQ + 1, N], F16, tag="KS", name="KS")
                nc.sync.dma_start(
                    KS[:].rearrange("c (g u) -> c g u", g=4),
                    kb_out[km][:].rearrange("g c u -> c g u"))
                # Q panel: rows 0:32 = q channels, row 32 = -m (written later)
                QS = prpool.tile([CQ + 1, SLAB], F16, tag="QS", name="QS")
                nc.vector.tensor_copy(QS[0:32, :], qk_sb[qm][0:32, :])
                # V^T panel for this direction, bf16, one DMA per gathered shard
                Vb = prpool.tile([128, 32, CD], BF16, tag="Vb", name="Vb")
                for g in range(4):
                    nc.sync.dma_start(
                        Vb[:, 8 * g:8 * g + 8, :],
                        vb_out[km][g].rearrange("(pc p) c -> p pc c", p=128))

                # pass A: subsampled row maxes of S -> -m into QS row 32.
                # stride-4 keys; slack is bounded (~65) and bf16 exp absorbs it
                mstack = epool.tile([128, 8], F32, tag="mstack")
                for ic in range(8):
                    mt = epool.tile([128, 2], F32, tag="mtmp")
                    for h in range(2):
                        psA = pp.tile([128, 512], F32, tag="ps")
                        nc.tensor.matmul(
                            psA[:],
                            QS[0:32, 128 * ic:128 * ic + 128],
                            KS[0:32].rearrange("c (u s) -> c u s", s=4)
                              [:, 512 * h:512 * h + 512, 0],
                            start=True, stop=True)
                        nc.vector.reduce_max(mt[:, h:h + 1], psA[:],
                                             axis=mybir.AxisListType.X)
                    nc.vector.reduce_max(mstack[:, ic:ic + 1], mt[:],
                                         axis=mybir.AxisListType.X)
                psT = pp.tile([128, 512], F32, tag="ps")
                nc.tensor.transpose(psT[0:8, 0:128], mstack[:], negI[:])
                mneg = epool.tile([8, 128], F16, tag="mneg")
                nc.vector.tensor_scalar_mul(mneg[:], psT[0:8, 0:128], -1.0)
                nc.sync.dma_start(QS[32:33, :], mneg[:])

                # software-pipelined flash loop over key chunks:
                # PE order S(0) S(1) O(0) S(2) O(1) ... so exp(t) on the
                # scalar engine overlaps S(t+1) on the PE
                o_sb = prpool.tile([128, 2, SLAB], F16, tag="osb", name="osb")
                for ic2 in range(2):
                    psO = [pp.tile([128, 512], F32, tag="ps", name=f"psO_{i}") for i in range(2)]
                    lacc = [epool.tile([128, 512], BF16, tag=f"lacc{i}",
                                       name=f"lacc{i}", bufs=2)
                            for i in range(2)]
                    prev = None
                    for t in range(32):
                        ps_st = pp.tile([128, 512], F32, tag="ps")
                        nc.tensor.matmul(
                            ps_st[:], KS[:, 128 * t:128 * t + 128],
                            QS[:, 512 * ic2:512 * ic2 + 512],
                            start=True, stop=True)
                        if prev is not None:
                            PTp, tp = prev
                            for mc in range(2):
                                nc.tensor.matmul(
                                    psO[mc][:],
                                    Vb[:, tp, 128 * mc:128 * mc + 128], PTp[:],
                                    start=(tp == 0), stop=False)
                            la = lacc[tp % 2]
                            if tp < 2:
                                nc.vector.tensor_copy(la[:], PTp[:])
                            else:
                                nc.vector.tensor_add(la[:], la[:], PTp[:])
                        PT = ptpool.tile([128, 512], BF16, tag="PT")
                        nc.scalar.activation(PT[:], ps_st[:], AF.Exp)
                        prev = (PT, t)
                    PTp, tp = prev
                    for mc in range(2):
                        nc.tensor.matmul(
                            psO[mc][:], Vb[:, tp, 128 * mc:128 * mc + 128],
                            PTp[:], start=False, stop=True)
                    nc.vector.tensor_add(lacc[1][:], lacc[1][:], PTp[:])
                    psl = pp.tile([128, 512], F32, tag="ps")
                    nc.tensor.matmul(psl[0:1, :], ones_b[:], lacc[0][:],
                                     start=True, stop=False)
                    nc.tensor.matmul(psl[0:1, :], ones_b[:], lacc[1][:],
                                     start=False, stop=True)
                    # epilogue: o = gamma*O/l + (conv + gamma*v_b)
                    recip = epool.tile([1, 512], F32, tag="recip")
                    nc.vector.reciprocal(recip[:], psl[0:1, :])
                    recg = epool.tile([1, 512], BF16, tag="recg")
                    nc.vector.tensor_scalar_mul(
                        recg[:], recip[:], sb[f"gamma_{km}"][0:1, 0:1])
                    psb = pp.tile([128, 512], F32, tag="ps")
                    nc.tensor.matmul(psb[:], onesc_b[:], recg[:],
                                     start=True, stop=True)
                    rb = epool.tile([128, 512], F32, tag="rb")
                    nc.scalar.activation(rb[:], psb[:], AF.Copy)
                    for mc in range(2):
                        t1 = epool.tile([128, 512], F32, tag="t1")
                        nc.vector.tensor_tensor(t1[:], psO[mc][:], rb[:],
                                                op=mybir.AluOpType.mult)
                        nc.vector.tensor_tensor(
                            o_sb[:, mc, 512 * ic2:512 * ic2 + 512], t1[:],
                            convb_sb[km][:, mc, 512 * ic2:512 * ic2 + 512],
                            op=mybir.AluOpType.add)

                # up-projection (fp16) + biases + input residual
                for oc in range(4):
                    for n2 in range(2):
                        psu = pp.tile([128, 512], F32, tag="ps")
                        for kc in range(2):
                            nc.tensor.matmul(
                                psu[:],
                                sb[f"upw_{km}"][:, kc, 128 * oc:128 * oc + 128],
                                o_sb[:, kc, 512 * n2:512 * n2 + 512],
                                start=(kc == 0), stop=(kc == 1))
                        tb = epool.tile([128, 512], F16, tag="tb")
                        nc.scalar.activation(tb[:], psu[:], AF.Identity,
                                             bias=sb[f"upb_{km}"][:, oc:oc + 1])
                        ob = epool.tile([128, 512], F32, tag="ob")
                        nc.vector.tensor_tensor(
                            ob[:], tb[:],
                            sb[f"xs_{km}"][:, oc, 1 + 8 * n2: 9 + 8 * n2,
                                           1:1 + W],
                            op=mybir.AluOpType.add)
                        nc.sync.dma_start(
                            OUT[km][128 * oc:128 * oc + 128,
                                    512 * n2:512 * n2 + 512], ob[:])

    nc.compile()
    return nc


@functools.lru_cache(maxsize=1)
def _program():
    return _build()


def _prep_shared(inputs):
    W_ = {}
    for m in MODS:
        cw = np.asarray(inputs[f"conv_w_{m}"], np.float32)       # [CD,CIN,3,3]
        W_[f"cw_{m}"] = np.ascontiguousarray(
            cw.transpose(1, 2, 3, 0).reshape(4, 128, 3, 3, CD)
              .transpose(2, 3, 0, 1, 4).reshape(9, 4, 128, CD)).astype(np.float16)
        g = np.asarray(inputs[f"bn_g_{m}"], np.float64)
        bb = np.asarray(inputs[f"bn_b_{m}"], np.float64)
        mu = np.asarray(inputs[f"bn_m_{m}"], np.float64)
        v = np.asarray(inputs[f"bn_v_{m}"], np.float64)
        cb = np.asarray(inputs[f"conv_b_{m}"], np.float64)
        scale = (g / np.sqrt(v + 1e-5))
        shift = bb - mu * scale + cb * scale     # fold conv bias into BN shift
        W_[f"bna_{m}"] = np.ascontiguousarray(
            scale.astype(np.float32).reshape(2, 128).T)
        W_[f"bnb_{m}"] = np.ascontiguousarray(
            shift.astype(np.float32).reshape(2, 128).T)
        W_[f"alpha_{m}"] = np.full((128, 1),
                                   np.float32(inputs[f"prelu_{m}"]), np.float32)
        W_[f"gamma_{m}"] = np.asarray(inputs[f"gamma_{m}"],
                                      np.float32).reshape(1, 1)
        qk = np.concatenate([np.asarray(inputs[f"q_w_{m}"], np.float32),
                             np.asarray(inputs[f"k_w_{m}"], np.float32)], 0)
        W_[f"qkw_{m}"] = np.ascontiguousarray(
            qk.T.reshape(2, 128, 64)).astype(np.float16)
        W_[f"qkb_{m}"] = np.concatenate(
            [np.asarray(inputs[f"q_b_{m}"], np.float32),
             np.asarray(inputs[f"k_b_{m}"], np.float32)], 0).reshape(64, 1)
        W_[f"vw_{m}"] = np.ascontiguousarray(
            np.asarray(inputs[f"v_w_{m}"], np.float32).T
            .reshape(2, 128, CD)).astype(np.float16)
        W_[f"upw_{m}"] = np.ascontiguousarray(
            np.asarray(inputs[f"up_w_{m}"], np.float32).T
            .reshape(2, 128, CIN)).astype(np.float16)
        W_[f"upb_{m}"] = np.ascontiguousarray(
            np.asarray(inputs[f"up_b_{m}"], np.float32).reshape(4, 128).T)
        gvb = (np.float32(inputs[f"gamma_{m}"])
               * np.asarray(inputs[f"v_b_{m}"], np.float32))
        W_[f"gvb_{m}"] = np.ascontiguousarray(gvb.reshape(2, 128).T)
    W_["negI"] = -np.eye(128, dtype=np.float32)
    return W_


def _slab(x_b, s):
    xp = np.zeros((CIN, HR, WP), np.float32)
    r0 = SLAB_ROWS * s - 1
    lo, hi = max(r0, 0), min(r0 + HR, H)
    xp[:, lo - r0:hi - r0, 1:1 + W] = x_b[:, lo:hi, :]
    return np.ascontiguousarray(
        xp.reshape(4, 128, HR, WP).transpose(1, 0, 2, 3)).astype(np.float16)


def kernel(**inputs):
    nc = _program()
    W_ = _prep_shared(inputs)
    xin = {m: np.asarray(inputs[f"input_{m}"], np.float32) for m in MODS}
    in_maps = []
    for cid in range(N_CORES):
        b, s = cid // 4, cid % 4
        im = dict(W_)
        for m in MODS:
            im[f"xs_{m}"] = _slab(xin[m][b], s)
        in_maps.append(im)
    res = run_bass_kernel_spmd(nc, in_maps, core_ids=list(range(N_CORES)))
    out = {m: np.zeros((B, CIN, H, W), np.float32) for m in MODS}
    for cid in range(N_CORES):
        b, s = cid // 4, cid % 4
        for m in MODS:
            out[m][b, :, SLAB_ROWS * s:SLAB_ROWS * (s + 1), :] = (
                res.results[cid][f"out_{m}"].reshape(CIN, SLAB_ROWS, W))
    return (out["rgb"], out["dsm"])


# revision 9
# speedup vs baseline: 1.0530x; 1.0530x over previous
"""Trainium2 Bass kernel for nn_CrossSemanticAttentionModule0 (cross-modal attention).

Sharding: 8 cores = (batch b in {0,1}) x (query/pixel slab s in {0..3}; 16 H-rows
= 1024 pixels each). Each core computes conv+BN+PReLU for its slab (with halo),
q/k/v projections, then one fused AllGather per modality carrying K (fp16, with
a folded-in ones row used as the softmax-max bias lane) and V^T (bf16) across
its 4-core batch group, then computes both cross-attentions for its query rows
over the full key axis and the up-projections + residuals for its output slab.

Precision: conv -> q/k -> S-logit chain runs in fp16 inputs with fp32 PSUM
accumulation; exp/P/V/O run in bf16 (needed for subsampled-max headroom: the
row max is estimated from every 4th key, max observed slack ~65 logits, and
bf16 holds e^65 easily). The flash loop is software-pipelined (S of chunk T+1
issues before O of chunk T) and exp is batched over double key-chunks.
"""

import numpy as np
import functools

import concourse.bass as bass
import concourse.mybir as mybir
import concourse.tile as tile
import concourse.bacc as bacc
from concourse.bass_utils import run_bass_kernel_spmd

B, CIN, H, W = 2, 512, 64, 64
CD, CQ = 256, 32
N = H * W                 # 4096 pixels
SLAB_ROWS = 16            # H rows per core
SLAB = SLAB_ROWS * W      # 1024 pixels per core
HR = SLAB_ROWS + 2        # halo rows
WP = W + 2                # padded width
N_CORES = 8
MODS = ("rgb", "dsm")
F32 = mybir.dt.float32
F16 = mybir.dt.float16
BF16 = mybir.dt.bfloat16
AF = mybir.ActivationFunctionType
RG = [[0, 1, 2, 3], [4, 5, 6, 7]]
KE = (CQ + 1) * SLAB      # fp16 elems of K block (incl. ones row)
KVE = KE + SLAB * CD      # total fp16 elems per rank in the fused AllGather


def _build():
    nc = bacc.Bacc("TRN2", target_bir_lowering=False, debug=False,
                   num_devices=N_CORES)

    D = {}
    def din(name, shape, dt):
        D[name] = nc.dram_tensor(name, shape, dt, kind="ExternalInput").ap()
    for m in MODS:
        din(f"xs_{m}", [128, 4, HR, WP], F16)
        din(f"cw_{m}", [9, 4, 128, CD], F16)
        din(f"bna_{m}", [128, 2], F32)
        din(f"bnb_{m}", [128, 2], F32)
        din(f"alpha_{m}", [128, 1], F32)
        din(f"gamma_{m}", [1, 1], F32)
        din(f"qkw_{m}", [2, 128, 64], F16)
        din(f"qkb_{m}", [64, 1], F32)
        din(f"vw_{m}", [2, 128, CD], F16)
        din(f"upw_{m}", [2, 128, CIN], F16)
        din(f"upb_{m}", [128, 4], F32)
        din(f"gvb_{m}", [128, 2], F32)
    din("negI", [128, 128], F32)
    OUT = {m: nc.dram_tensor(f"out_{m}", [CIN, SLAB], F32,
                             kind="ExternalOutput").ap() for m in MODS}

    with tile.TileContext(nc) as tc:
        with (
            tc.tile_pool(name="const", bufs=1) as cpool,
            tc.tile_pool(name="cw", bufs=3) as cwpool,
            tc.tile_pool(name="big", bufs=1) as bpool,
            tc.tile_pool(name="pair", bufs=2) as prpool,
            tc.tile_pool(name="pt", bufs=4) as ptpool,
            tc.tile_pool(name="eps", bufs=2) as epool,
            tc.tile_pool(name="ps", bufs=4, space="PSUM") as pp,
            tc.tile_pool(name="ps2", bufs=2, space="PSUM") as pp2,
            tc.tile_pool(name="dram", bufs=1, space="DRAM") as dpool,
        ):
            sb = {}
            # rgb input slab + first conv weights come first: conv rgb is the
            # head of the critical path
            for m in MODS:
                sb[f"xs_{m}"] = cpool.tile([128, 4, HR, WP], F16,
                                           tag=f"xs_{m}", name=f"xs_{m}")
            nc.sync.dma_start(sb["xs_rgb"][:, 0:2], D["xs_rgb"][:, 0:2])
            nc.sync.dma_start(sb["xs_rgb"][:, 2:4], D["xs_rgb"][:, 2:4])

            def load_smalls(m):
                for nm, shp, dt in (
                    (f"bna_{m}", [128, 2], F32),
                    (f"bnb_{m}", [128, 2], F32),
                    (f"alpha_{m}", [128, 1], F32),
                    (f"gamma_{m}", [1, 1], F32),
                    (f"qkw_{m}", [128, 2, 64], F16),
                    (f"qkb_{m}", [64, 1], F32),
                    (f"vw_{m}", [128, 2, CD], F16),
                    (f"upw_{m}", [128, 2, CIN], F16),
                    (f"upb_{m}", [128, 4], F32),
                    (f"gvb_{m}", [128, 2], F32),
                ):
                    t = cpool.tile(shp, dt, tag=nm, name=nm)
                    src = D[nm]
                    if nm.startswith(("qkw", "vw", "upw")):
                        src = src.rearrange("k p f -> p k f", p=128)
                    nc.sync.dma_start(t[:], src)
                    sb[nm] = t

            negI = cpool.tile([128, 128], F32, tag="negI")
            ones_b = cpool.tile([128, 1], BF16, tag="ones_b")
            nc.vector.memset(ones_b[:], 1.0)
            onesc_b = cpool.tile([1, 128], BF16, tag="onesc_b")
            nc.vector.memset(onesc_b[:], 1.0)
            onesK = cpool.tile([1, SLAB], F16, tag="onesK")
            nc.vector.memset(onesK[:], 1.0)

            # DRAM bounce buffers: one fused K+V AllGather per modality
            kvb_in, kvb_out = {}, {}
            for m in MODS:
                kvb_in[m] = dpool.tile([KVE], F16, tag=f"kvi_{m}", name=f"kvi_{m}")
                kvb_out[m] = dpool.tile([4, KVE], F16, tag=f"kvo_{m}", name=f"kvo_{m}")
                nc.sync.dma_start(kvb_in[m][CQ * SLAB:KE], onesK[:])

            conv_sb, convb_sb, qk_sb, vt_sb = {}, {}, {}, {}

            # ---- per-modality: conv -> bn+prelu -> q/k/v projections ----
            for mi, m in enumerate(MODS):
                xs = sb[f"xs_{m}"]
                conv_sb[m] = bpool.tile([128, 2, SLAB], F16, tag=f"conv_{m}", name=f"conv_{m}")
                convb_sb[m] = bpool.tile([128, 2, SLAB], F32, tag=f"convb_{m}", name=f"convb_{m}")
                qk_sb[m] = bpool.tile([64, SLAB], F16, tag=f"qk_{m}", name=f"qk_{m}")
                vt_sb[m] = bpool.tile([128, 8, CD], BF16, tag=f"vt_{m}", name=f"vt_{m}")

                pcv = [[None, None], [None, None]]
                for mc in range(2):
                    for n2 in range(2):
                        pcv[mc][n2] = pp.tile([128, 512], F32, tag="ps", name=f"pcv_{mc}_{n2}")
                for tap in range(9):
                    dy, dx = tap // 3, tap % 3
                    cwt = cwpool.tile([128, 4, CD], F16, tag="cwt")
                    nc.sync.dma_start(
                        cwt[:], D[f"cw_{m}"][tap].rearrange("k p f -> p k f", p=128))
                    if tap == 0:
                        # overlap the rest of the startup traffic with conv
                        if mi == 0:
                            nc.sync.dma_start(sb["xs_dsm"][:, 0:2],
                                              D["xs_dsm"][:, 0:2])
                            nc.sync.dma_start(sb["xs_dsm"][:, 2:4],
                                              D["xs_dsm"][:, 2:4])
                            nc.sync.dma_start(negI[:], D["negI"])
                        load_smalls(m)
                    for kc in range(4):
                        for mc in range(2):
                            for n2 in range(2):
                                nc.tensor.matmul(
                                    pcv[mc][n2][:],
                                    cwt[:, kc, 128 * mc:128 * mc + 128],
                                    xs[:, kc, dy + 8 * n2: dy + 8 * n2 + 8,
                                       dx:dx + W],
                                    start=(tap == 0 and kc == 0),
                                    stop=(tap == 8 and kc == 3),
                                )
                for mc in range(2):
                    for n2 in range(2):
                        nc.scalar.activation(
                            conv_sb[m][:, mc, 512 * n2:512 * n2 + 512],
                            pcv[mc][n2][:], AF.Prelu,
                            bias=sb[f"bnb_{m}"][:, mc:mc + 1],
                            scale=sb[f"bna_{m}"][:, mc:mc + 1],
                            alpha=sb[f"alpha_{m}"][:, 0:1],
                        )
                # conv + gamma*v_b (residual-with-v-bias, exact through softmax)
                for mc in range(2):
                    nc.scalar.activation(
                        convb_sb[m][:, mc, :], conv_sb[m][:, mc, :],
                        AF.Identity, bias=sb[f"gvb_{m}"][:, mc:mc + 1])

                # q/k projections (64 = [q;k] channels)
                for n2 in range(2):
                    ps = pp.tile([128, 512], F32, tag="ps")
                    for kc in range(2):
                        nc.tensor.matmul(
                            ps[0:64, :], sb[f"qkw_{m}"][:, kc, :],
                            conv_sb[m][:, kc, 512 * n2:512 * n2 + 512],
                            start=(kc == 0), stop=(kc == 1))
                    nc.vector.tensor_scalar_add(
                        qk_sb[m][0:64, 512 * n2:512 * n2 + 512], ps[0:64, :],
                        sb[f"qkb_{m}"][:, 0:1])
                nc.sync.dma_start(
                    kvb_in[m][0:CQ * SLAB].rearrange("(c u) -> c u", u=SLAB),
                    qk_sb[m][32:64, :])

                # V^T projection ([pix, c] layout, bf16; v bias handled via gvb)
                for pc in range(8):
                    ps = pp.tile([128, 512], F32, tag="ps")
                    for kc in range(2):
                        nc.tensor.matmul(
                            ps[:, 0:CD],
                            conv_sb[m][:, kc, 128 * pc:128 * pc + 128],
                            sb[f"vw_{m}"][:, kc, :],
                            start=(kc == 0), stop=(kc == 1))
                    nc.vector.tensor_copy(vt_sb[m][:, pc, :], ps[:, 0:CD])
                nc.sync.dma_start(
                    kvb_in[m][KE:KVE].rearrange("(pc p c) -> p pc c",
                                                p=128, c=CD),
                    vt_sb[m][:].bitcast(F16))
                nc.gpsimd.collective_compute(
                    "AllGather", mybir.AluOpType.bypass, replica_groups=RG,
                    ins=[kvb_in[m].opt()], outs=[kvb_out[m].opt()])

            # ---- attention pairs: (query mod, key/value mod) ----
            for qm, km in (("dsm", "rgb"), ("rgb", "dsm")):
                # K panel: rows 0:32 = K channels, row 32 = ones (bias row)
                KS = prpool.tile([CQ + 1, N], F16, tag="KS", name="KS")
                nc.sync.dma_start(
                    KS[:].rearrange("c (g u) -> c g u", g=4),
                    kvb_out[km][:, 0:KE].rearrange("g (c u) -> c g u", u=SLAB))
                # Q panel: rows 0:32 = q channels, row 32 = -m (written later)
                QS = prpool.tile([CQ + 1, SLAB], F16, tag="QS", name="QS")
                nc.vector.tensor_copy(QS[0:32, :], qk_sb[qm][0:32, :])
                # V^T panel for this direction, bf16, one DMA per gathered shard
                Vb = prpool.tile([128, 32, CD], BF16, tag="Vb", name="Vb")
                for g in range(4):
                    nc.sync.dma_start(
                        Vb[:, 8 * g:8 * g + 8, :],
                        kvb_out[km][g, KE:KVE].rearrange(
                            "(pc p c) -> p pc c", p=128, c=CD).bitcast(BF16))

                # pass A: subsampled row maxes of S -> -m into QS row 32.
                # stride-4 keys; slack is bounded (~65) and bf16 exp absorbs it
                mstack = epool.tile([128, 8], F32, tag="mstack")
                for ic in range(8):
                    mt = epool.tile([128, 2], F32, tag="mtmp")
                    for h in range(2):
                        psA = pp.tile([128, 512], F32, tag="ps")
                        nc.tensor.matmul(
                            psA[:],
                            QS[0:32, 128 * ic:128 * ic + 128],
                            KS[0:32].rearrange("c (u s) -> c u s", s=4)
                              [:, 512 * h:512 * h + 512, 0],
                            start=True, stop=True)
                        nc.vector.reduce_max(mt[:, h:h + 1], psA[:],
                                             axis=mybir.AxisListType.X)
                    nc.vector.reduce_max(mstack[:, ic:ic + 1], mt[:],
                                         axis=mybir.AxisListType.X)
                psT = pp.tile([128, 512], F32, tag="ps")
                nc.tensor.transpose(psT[0:8, 0:128], mstack[:], negI[:])
                mneg = epool.tile([8, 128], F16, tag="mneg")
                nc.vector.tensor_scalar_mul(mneg[:], psT[0:8, 0:128], -1.0)
                nc.sync.dma_start(QS[32:33, :], mneg[:])

                # software-pipelined flash loop over double key-chunks:
                # PE order S(0) S(1) O(0) S(2) O(1) ... so exp on the scalar
                # engine always overlaps S-matmuls on the PE
                o_sb = prpool.tile([128, 2, SLAB], F16, tag="osb", name="osb")
                for ic2 in range(2):
                    psO = [pp.tile([128, 512], F32, tag="ps", name=f"psO_{i}") for i in range(2)]
                    lacc = [epool.tile([128, 512], BF16, tag=f"lacc{i}",
                                       name=f"lacc{i}", bufs=2)
                            for i in range(2)]

                    def drain(PT2, T):
                        for j in range(2):
                            t = 2 * T + j
                            for mc in range(2):
                                nc.tensor.matmul(
                                    psO[mc][:],
                                    Vb[:, t, 128 * mc:128 * mc + 128],
                                    PT2[:, j, :],
                                    start=(t == 0), stop=(t == 31))
                            eng = nc.vector if j == 0 else nc.gpsimd
                            if T == 0:
                                eng.tensor_copy(lacc[j][:], PT2[:, j, :])
                            else:
                                eng.tensor_add(lacc[j][:], lacc[j][:],
                                               PT2[:, j, :])

                    prev = None
                    for T in range(16):
                        ps2 = pp2.tile([128, 2, 512], F32, tag="ps2")
                        for j in range(2):
                            nc.tensor.matmul(
                                ps2[:, j, :],
                                KS[:, 128 * (2 * T + j):128 * (2 * T + j) + 128],
                                QS[:, 512 * ic2:512 * ic2 + 512],
                                start=True, stop=True)
                        if prev is not None:
                            drain(*prev)
                        PT2 = ptpool.tile([128, 2, 512], BF16, tag="PT")
                        nc.scalar.activation(PT2[:], ps2[:], AF.Exp)
                        prev = (PT2, T)
                    drain(*prev)

                    psl = pp.tile([128, 512], F32, tag="ps")
                    nc.tensor.matmul(psl[0:1, :], ones_b[:], lacc[0][:],
                                     start=True, stop=False)
                    nc.tensor.matmul(psl[0:1, :], ones_b[:], lacc[1][:],
                                     start=False, stop=True)
                    # epilogue: o = gamma*O/l + (conv + gamma*v_b)
                    recip = epool.tile([1, 512], F32, tag="recip")
                    nc.vector.reciprocal(recip[:], psl[0:1, :])
                    recg = epool.tile([1, 512], BF16, tag="recg")
                    nc.vector.tensor_scalar_mul(
                        recg[:], recip[:], sb[f"gamma_{km}"][0:1, 0:1])
                    psb = pp.tile([128, 512], F32, tag="ps")
                    nc.tensor.matmul(psb[:], onesc_b[:], recg[:],
                                     start=True, stop=True)
                    rb = epool.tile([128, 512], F32, tag="rb")
                    nc.vector.tensor_copy(rb[:], psb[:])
                    for mc in range(2):
                        t1 = epool.tile([128, 512], F32, tag="t1")
                        nc.vector.tensor_tensor(t1[:], psO[mc][:], rb[:],
                                                op=mybir.AluOpType.mult)
                        nc.vector.tensor_tensor(
                            o_sb[:, mc, 512 * ic2:512 * ic2 + 512], t1[:],
                            convb_sb[km][:, mc, 512 * ic2:512 * ic2 + 512],
                            op=mybir.AluOpType.add)

                # up-projection (fp16) + biases + input residual
                for oc in range(4):
                    for n2 in range(2):
                        psu = pp.tile([128, 512], F32, tag="ps")
                        for kc in range(2):
                            nc.tensor.matmul(
                                psu[:],
                                sb[f"upw_{km}"][:, kc, 128 * oc:128 * oc + 128],
                                o_sb[:, kc, 512 * n2:512 * n2 + 512],
                                start=(kc == 0), stop=(kc == 1))
                        tb = epool.tile([128, 512], F16, tag="tb")
                        nc.vector.tensor_scalar_add(
                            tb[:], psu[:], sb[f"upb_{km}"][:, oc:oc + 1])
                        ob = epool.tile([128, 512], F32, tag="ob")
                        nc.vector.tensor_tensor(
                            ob[:], tb[:],
                            sb[f"xs_{km}"][:, oc, 1 + 8 * n2: 9 + 8 * n2,
                                           1:1 + W],
                            op=mybir.AluOpType.add)
                        nc.sync.dma_start(
                            OUT[km][128 * oc:128 * oc + 128,
                                    512 * n2:512 * n2 + 512], ob[:])

    nc.compile()
    return nc


@functools.lru_cache(maxsize=1)
def _program():
    return _build()


def _prep_shared(inputs):
    W_ = {}
    for m in MODS:
        cw = np.asarray(inputs[f"conv_w_{m}"], np.float32)       # [CD,CIN,3,3]
        W_[f"cw_{m}"] = np.ascontiguousarray(
            cw.transpose(1, 2, 3, 0).reshape(4, 128, 3, 3, CD)
              .transpose(2, 3, 0, 1, 4).reshape(9, 4, 128, CD)).astype(np.float16)
        g = np.asarray(inputs[f"bn_g_{m}"], np.float64)
        bb = np.asarray(inputs[f"bn_b_{m}"], np.float64)
        mu = np.asarray(inputs[f"bn_m_{m}"], np.float64)
        v = np.asarray(inputs[f"bn_v_{m}"], np.float64)
        cb = np.asarray(inputs[f"conv_b_{m}"], np.float64)
        scale = (g / np.sqrt(v + 1e-5))
        shift = bb - mu * scale + cb * scale     # fold conv bias into BN shift
        W_[f"bna_{m}"] = np.ascontiguousarray(
            scale.astype(np.float32).reshape(2, 128).T)
        W_[f"bnb_{m}"] = np.ascontiguousarray(
            shift.astype(np.float32).reshape(2, 128).T)
        W_[f"alpha_{m}"] = np.full((128, 1),
                                   np.float32(inputs[f"prelu_{m}"]), np.float32)
        W_[f"gamma_{m}"] = np.asarray(inputs[f"gamma_{m}"],
                                      np.float32).reshape(1, 1)
        qk = np.concatenate([np.asarray(inputs[f"q_w_{m}"], np.float32),
                             np.asarray(inputs[f"k_w_{m}"], np.float32)], 0)
        W_[f"qkw_{m}"] = np.ascontiguousarray(
            qk.T.reshape(2, 128, 64)).astype(np.float16)
        W_[f"qkb_{m}"] = np.concatenate(
            [np.asarray(inputs[f"q_b_{m}"], np.float32),
             np.asarray(inputs[f"k_b_{m}"], np.float32)], 0).reshape(64, 1)
        W_[f"vw_{m}"] = np.ascontiguousarray(
            np.asarray(inputs[f"v_w_{m}"], np.float32).T
            .reshape(2, 128, CD)).astype(np.float16)
        W_[f"upw_{m}"] = np.ascontiguousarray(
            np.asarray(inputs[f"up_w_{m}"], np.float32).T
            .reshape(2, 128, CIN)).astype(np.float16)
        W_[f"upb_{m}"] = np.ascontiguousarray(
            np.asarray(inputs[f"up_b_{m}"], np.float32).reshape(4, 128).T)
        gvb = (np.float32(inputs[f"gamma_{m}"])
               * np.asarray(inputs[f"v_b_{m}"], np.float32))
        W_[f"gvb_{m}"] = np.ascontiguousarray(gvb.reshape(2, 128).T)
    W_["negI"] = -np.eye(128, dtype=np.float32)
    return W_


def _slab(x_b, s):
    xp = np.zeros((CIN, HR, WP), np.float32)
    r0 = SLAB_ROWS * s - 1
    lo, hi = max(r0, 0), min(r0 + HR, H)
    xp[:, lo - r0:hi - r0, 1:1 + W] = x_b[:, lo:hi, :]
    return np.ascontiguousarray(
        xp.reshape(4, 128, HR, WP).transpose(1, 0, 2, 3)).astype(np.float16)


def kernel(**inputs):
    nc = _program()
    W_ = _prep_shared(inputs)
    xin = {m: np.asarray(inputs[f"input_{m}"], np.float32) for m in MODS}
    in_maps = []
    for cid in range(N_CORES):
        b, s = cid // 4, cid % 4
        im = dict(W_)
        for m in MODS:
            im[f"xs_{m}"] = _slab(xin[m][b], s)
        in_maps.append(im)
    res = run_bass_kernel_spmd(nc, in_maps, core_ids=list(range(N_CORES)))
    out = {m: np.zeros((B, CIN, H, W), np.float32) for m in MODS}
    for cid in range(N_CORES):
        b, s = cid // 4, cid % 4
        for m in MODS:
            out[m][b, :, SLAB_ROWS * s:SLAB_ROWS * (s + 1), :] = (
                res.results[cid][f"out_{m}"].reshape(CIN, SLAB_ROWS, W))
    return (out["rgb"], out["dsm"])


# revision 12
# speedup vs baseline: 1.1199x; 1.0635x over previous
"""Trainium2 Bass kernel for nn_CrossSemanticAttentionModule0 (cross-modal attention).

Sharding: 8 cores = (batch b in {0,1}) x (query/pixel slab s in {0..3}; 16 H-rows
= 1024 pixels each). Each core computes conv+BN+PReLU for its slab (with halo),
q/k/v projections, then one fused AllGather per modality carrying K (fp16, with
a folded-in ones row used as the softmax-max bias lane) and V^T (bf16) across
its 4-core batch group, then computes both cross-attentions for its query rows
over the full key axis and the up-projections + residuals for its output slab.

Precision: conv -> q/k -> S-logit chain runs in fp16 inputs with fp32 PSUM
accumulation; exp/P/V/O run in bf16 (needed for subsampled-max headroom: the
row max is estimated from every 4th key, max observed slack ~65 logits, and
bf16 holds e^65 easily). The flash loop is software-pipelined (S of chunk T+1
issues before O of chunk T) and exp is batched over double key-chunks.
"""

import numpy as np
import functools

import concourse.bass as bass
import concourse.mybir as mybir
import concourse.tile as tile
import concourse.bacc as bacc
from concourse.bass_utils import run_bass_kernel_spmd

B, CIN, H, W = 2, 512, 64, 64
CD, CQ = 256, 32
N = H * W                 # 4096 pixels
SLAB_ROWS = 16            # H rows per core
SLAB = SLAB_ROWS * W      # 1024 pixels per core
HR = SLAB_ROWS + 2        # halo rows
WP = W + 2                # padded width
N_CORES = 8
MODS = ("rgb", "dsm")
F32 = mybir.dt.float32
F16 = mybir.dt.float16
BF16 = mybir.dt.bfloat16
AF = mybir.ActivationFunctionType
RG = [[0, 1, 2, 3], [4, 5, 6, 7]]
KE = (CQ + 1) * SLAB      # fp16 elems of K block (incl. ones row)
KVE = KE + SLAB * CD      # total fp16 elems per rank in the fused AllGather


def _build():
    nc = bacc.Bacc("TRN2", target_bir_lowering=False, debug=False,
                   num_devices=N_CORES)

    D = {}
    def din(name, shape, dt):
        D[name] = nc.dram_tensor(name, shape, dt, kind="ExternalInput").ap()
    for m in MODS:
        din(f"xs_{m}", [128, 3, 4, HR, 64], F16)
        din(f"cw_{m}", [9, 4, 128, CD], F16)
        din(f"bna_{m}", [128, 2], F32)
        din(f"bnb_{m}", [128, 2], F32)
        din(f"alpha_{m}", [128, 1], F32)
        din(f"gamma_{m}", [1, 1], F32)
        din(f"qkw_{m}", [2, 128, 64], F16)
        din(f"qkb_{m}", [64, 1], F32)
        din(f"vw_{m}", [2, 128, CD], F16)
        din(f"upw_{m}", [2, 128, CIN], F16)
        din(f"upb_{m}", [128, 4], F32)
        din(f"gvb_{m}", [128, 2], F32)
    din("negI", [128, 128], F32)
    OUT = {m: nc.dram_tensor(f"out_{m}", [CIN, SLAB], F32,
                             kind="ExternalOutput").ap() for m in MODS}

    with tile.TileContext(nc) as tc:
        with (
            tc.tile_pool(name="const", bufs=1) as cpool,
            tc.tile_pool(name="cw", bufs=3) as cwpool,
            tc.tile_pool(name="big", bufs=1) as bpool,
            tc.tile_pool(name="pair", bufs=2) as prpool,
            tc.tile_pool(name="pt", bufs=4) as ptpool,
            tc.tile_pool(name="eps", bufs=2) as epool,
            tc.tile_pool(name="ps", bufs=4, space="PSUM") as pp,
            tc.tile_pool(name="ps2", bufs=2, space="PSUM") as pp2,
            tc.tile_pool(name="dram", bufs=1, space="DRAM") as dpool,
        ):
            sb = {}
            # rgb input slab + first conv weights come first: conv rgb is the
            # head of the critical path
            for m in MODS:
                sb[f"xs_{m}"] = cpool.tile([128, 3, 4, HR, 64], F16,
                                           tag=f"xs_{m}", name=f"xs_{m}")
            nc.sync.dma_start(sb["xs_rgb"][:, :, 0:2], D["xs_rgb"][:, :, 0:2])
            nc.sync.dma_start(sb["xs_rgb"][:, :, 2:4], D["xs_rgb"][:, :, 2:4])

            def load_smalls(m):
                for nm, shp, dt in (
                    (f"bna_{m}", [128, 2], F32),
                    (f"bnb_{m}", [128, 2], F32),
                    (f"alpha_{m}", [128, 1], F32),
                    (f"gamma_{m}", [1, 1], F32),
                    (f"qkw_{m}", [128, 2, 64], F16),
                    (f"qkb_{m}", [64, 1], F32),
                    (f"vw_{m}", [128, 2, CD], F16),
                    (f"upw_{m}", [128, 2, CIN], F16),
                    (f"upb_{m}", [128, 4], F32),
                    (f"gvb_{m}", [128, 2], F32),
                ):
                    t = cpool.tile(shp, dt, tag=nm, name=nm)
                    src = D[nm]
                    if nm.startswith(("qkw", "vw", "upw")):
                        src = src.rearrange("k p f -> p k f", p=128)
                    nc.sync.dma_start(t[:], src)
                    sb[nm] = t

            negI = cpool.tile([128, 128], F32, tag="negI")
            ones_b = cpool.tile([128, 1], BF16, tag="ones_b")
            nc.vector.memset(ones_b[:], 1.0)
            onesc_b = cpool.tile([1, 128], BF16, tag="onesc_b")
            nc.vector.memset(onesc_b[:], 1.0)
            onesK = cpool.tile([1, SLAB], F16, tag="onesK")
            nc.vector.memset(onesK[:], 1.0)

            # DRAM bounce buffers: one fused K+V AllGather per modality
            kvb_in, kvb_out = {}, {}
            for m in MODS:
                kvb_in[m] = dpool.tile([KVE], F16, tag=f"kvi_{m}", name=f"kvi_{m}")
                kvb_out[m] = dpool.tile([4, KVE], F16, tag=f"kvo_{m}", name=f"kvo_{m}")
                nc.sync.dma_start(kvb_in[m][CQ * SLAB:KE], onesK[:])

            conv_sb, convb_sb, qk_sb, vt_sb = {}, {}, {}, {}

            # ---- per-modality: conv -> bn+prelu -> q/k/v projections ----
            for mi, m in enumerate(MODS):
                xs = sb[f"xs_{m}"]
                conv_sb[m] = bpool.tile([128, 2, SLAB], F16, tag=f"conv_{m}", name=f"conv_{m}")
                convb_sb[m] = bpool.tile([128, 2, SLAB], F32, tag=f"convb_{m}", name=f"convb_{m}")
                qk_sb[m] = bpool.tile([64, SLAB], F16, tag=f"qk_{m}", name=f"qk_{m}")
                vt_sb[m] = bpool.tile([128, 8, CD], BF16, tag=f"vt_{m}", name=f"vt_{m}")

                pcv = [[None, None], [None, None]]
                for mc in range(2):
                    for n2 in range(2):
                        pcv[mc][n2] = pp.tile([128, 512], F32, tag="ps", name=f"pcv_{mc}_{n2}")
                for tap in range(9):
                    dy, dx = tap // 3, tap % 3
                    cwt = cwpool.tile([128, 4, CD], F16, tag="cwt")
                    nc.sync.dma_start(
                        cwt[:], D[f"cw_{m}"][tap].rearrange("k p f -> p k f", p=128))
                    if tap == 0:
                        # overlap the rest of the startup traffic with conv
                        if mi == 0:
                            nc.sync.dma_start(sb["xs_dsm"][:, :, 0:2],
                                              D["xs_dsm"][:, :, 0:2])
                            nc.sync.dma_start(sb["xs_dsm"][:, :, 2:4],
                                              D["xs_dsm"][:, :, 2:4])
                            nc.sync.dma_start(negI[:], D["negI"])
                        load_smalls(m)
                    for kc in range(4):
                        for mc in range(2):
                            for n2 in range(2):
                                nc.tensor.matmul(
                                    pcv[mc][n2][:],
                                    cwt[:, kc, 128 * mc:128 * mc + 128],
                                    xs[:, dx, kc,
                                       dy + 8 * n2: dy + 8 * n2 + 8, :],
                                    start=(tap == 0 and kc == 0),
                                    stop=(tap == 8 and kc == 3),
                                )
                for mc in range(2):
                    for n2 in range(2):
                        nc.scalar.activation(
                            conv_sb[m][:, mc, 512 * n2:512 * n2 + 512],
                            pcv[mc][n2][:], AF.Prelu,
                            bias=sb[f"bnb_{m}"][:, mc:mc + 1],
                            scale=sb[f"bna_{m}"][:, mc:mc + 1],
                            alpha=sb[f"alpha_{m}"][:, 0:1],
                        )
                # conv + gamma*v_b (residual-with-v-bias, exact through softmax)
                for mc in range(2):
                    nc.scalar.activation(
                        convb_sb[m][:, mc, :], conv_sb[m][:, mc, :],
                        AF.Identity, bias=sb[f"gvb_{m}"][:, mc:mc + 1])

                # q/k projections (64 = [q;k] channels)
                for n2 in range(2):
                    ps = pp.tile([128, 512], F32, tag="ps")
                    for kc in range(2):
                        nc.tensor.matmul(
                            ps[0:64, :], sb[f"qkw_{m}"][:, kc, :],
                            conv_sb[m][:, kc, 512 * n2:512 * n2 + 512],
                            start=(kc == 0), stop=(kc == 1))
                    nc.vector.tensor_scalar_add(
                        qk_sb[m][0:64, 512 * n2:512 * n2 + 512], ps[0:64, :],
                        sb[f"qkb_{m}"][:, 0:1])
                nc.sync.dma_start(
                    kvb_in[m][0:CQ * SLAB].rearrange("(c u) -> c u", u=SLAB),
                    qk_sb[m][32:64, :])

                # V^T projection ([pix, c] layout, bf16; v bias handled via gvb)
                for pc in range(8):
                    ps = pp.tile([128, 512], F32, tag="ps")
                    for kc in range(2):
                        nc.tensor.matmul(
                            ps[:, 0:CD],
                            conv_sb[m][:, kc, 128 * pc:128 * pc + 128],
                            sb[f"vw_{m}"][:, kc, :],
                            start=(kc == 0), stop=(kc == 1))
                    nc.vector.tensor_copy(vt_sb[m][:, pc, :], ps[:, 0:CD])
                nc.sync.dma_start(
                    kvb_in[m][KE:KVE].rearrange("(pc p c) -> p pc c",
                                                p=128, c=CD),
                    vt_sb[m][:].bitcast(F16))
                nc.gpsimd.collective_compute(
                    "AllGather", mybir.AluOpType.bypass, replica_groups=RG,
                    ins=[kvb_in[m].opt()], outs=[kvb_out[m].opt()])

            # ---- attention pairs: (query mod, key/value mod) ----
            for qm, km in (("dsm", "rgb"), ("rgb", "dsm")):
                # K panel, triplicated so the S-matmul contraction is 97
                # rows (>64 rows streams at 1 cyc/row; <=64 takes 2): rows
                # 0:96 are three copies of the 32 K channels (32-aligned
                # bases), row 96 is the gathered ones row; with -3m in QS row
                # 96 the matmul yields 3*S - 3m, undone by exp scale=1/3.
                KS = prpool.tile([3 * CQ + 1, N], F16, tag="KS", name="KS")
                kview = kvb_out[km][:, 0:KE].rearrange("g (c u) -> c g u",
                                                       u=SLAB)
                for r in range(3):
                    nc.sync.dma_start(
                        KS[32 * r:32 * r + 32].rearrange(
                            "c (g u) -> c g u", g=4), kview[0:32])
                nc.sync.dma_start(
                    KS[96:97].rearrange("c (g u) -> c g u", g=4), kview[32:33])
                QS = prpool.tile([3 * CQ + 1, SLAB], F16, tag="QS", name="QS")
                for r in range(3):
                    nc.vector.tensor_copy(QS[32 * r:32 * r + 32, :],
                                          qk_sb[qm][0:32, :])
                # V^T panel for this direction, bf16, one DMA per gathered shard
                Vb = prpool.tile([128, 32, CD], BF16, tag="Vb", name="Vb")
                for g in range(4):
                    nc.sync.dma_start(
                        Vb[:, 8 * g:8 * g + 8, :],
                        kvb_out[km][g, KE:KVE].rearrange(
                            "(pc p c) -> p pc c", p=128, c=CD).bitcast(BF16))

                # pass A: subsampled row maxes of S -> -m into QS row 32.
                # stride-4 keys; slack is bounded (~65) and bf16 exp absorbs it
                mstack = epool.tile([128, 8], F32, tag="mstack")
                for ic in range(8):
                    mt = epool.tile([128, 2], F32, tag="mtmp")
                    for h in range(2):
                        psA = pp.tile([128, 512], F32, tag="ps")
                        nc.tensor.matmul(
                            psA[:],
                            QS[0:32, 128 * ic:128 * ic + 128],
                            KS[0:32].rearrange("c (u s) -> c u s", s=4)
                              [:, 512 * h:512 * h + 512, 0],
                            start=True, stop=True)
                        nc.vector.reduce_max(mt[:, h:h + 1], psA[:],
                                             axis=mybir.AxisListType.X)
                    nc.vector.reduce_max(mstack[:, ic:ic + 1], mt[:],
                                         axis=mybir.AxisListType.X)
                psT = pp.tile([128, 512], F32, tag="ps")
                nc.tensor.transpose(psT[0:8, 0:128], mstack[:], negI[:])
                mneg = epool.tile([8, 128], F16, tag="mneg")
                nc.vector.tensor_scalar_mul(mneg[:], psT[0:8, 0:128], -3.0)
                nc.sync.dma_start(QS[96:97, :], mneg[:])

                # software-pipelined flash loop over double key-chunks:
                # PE order S(0) S(1) O(0) S(2) O(1) ... so exp on the scalar
                # engine always overlaps S-matmuls on the PE
                o_sb = prpool.tile([128, 2, SLAB], F16, tag="osb", name="osb")
                for ic2 in range(2):
                    psO = [pp.tile([128, 512], F32, tag="ps", name=f"psO_{i}") for i in range(2)]
                    lacc = [epool.tile([128, 512], BF16, tag=f"lacc{i}",
                                       name=f"lacc{i}", bufs=2)
                            for i in range(2)]

                    def drain(PT2, T):
                        for j in range(2):
                            t = 2 * T + j
                            for mc in range(2):
                                nc.tensor.matmul(
                                    psO[mc][:],
                                    Vb[:, t, 128 * mc:128 * mc + 128],
                                    PT2[:, j, :],
                                    start=(t == 0), stop=(t == 31))
                            eng = nc.vector if j == 0 else nc.gpsimd
                            if T == 0:
                                eng.tensor_copy(lacc[j][:], PT2[:, j, :])
                            else:
                                eng.tensor_add(lacc[j][:], lacc[j][:],
                                               PT2[:, j, :])

                    prev = None
                    for T in range(16):
                        ps2 = pp2.tile([128, 2, 512], F32, tag="ps2")
                        for j in range(2):
                            nc.tensor.matmul(
                                ps2[:, j, :],
                                KS[:, 128 * (2 * T + j):128 * (2 * T + j) + 128],
                                QS[:, 512 * ic2:512 * ic2 + 512],
                                start=True, stop=True)
                        if prev is not None:
                            drain(*prev)
                        PT2 = ptpool.tile([128, 2, 512], BF16, tag="PT")
                        nc.scalar.activation(PT2[:], ps2[:], AF.Exp, scale=1.0 / 3.0)
                        prev = (PT2, T)
                    drain(*prev)

                    psl = pp.tile([128, 512], F32, tag="ps")
                    nc.tensor.matmul(psl[0:1, :], ones_b[:], lacc[0][:],
                                     start=True, stop=False)
                    nc.tensor.matmul(psl[0:1, :], ones_b[:], lacc[1][:],
                                     start=False, stop=True)
                    # epilogue: o = gamma*O/l + (conv + gamma*v_b)
                    recip = epool.tile([1, 512], F32, tag="recip")
                    nc.vector.reciprocal(recip[:], psl[0:1, :])
                    recg = epool.tile([1, 512], BF16, tag="recg")
                    nc.vector.tensor_scalar_mul(
                        recg[:], recip[:], sb[f"gamma_{km}"][0:1, 0:1])
                    psb = pp.tile([128, 512], F32, tag="ps")
                    nc.tensor.matmul(psb[:], onesc_b[:], recg[:],
                                     start=True, stop=True)
                    rb = epool.tile([128, 512], F32, tag="rb")
                    nc.vector.tensor_copy(rb[:], psb[:])
                    for mc in range(2):
                        t1 = epool.tile([128, 512], F32, tag="t1")
                        nc.vector.tensor_tensor(t1[:], psO[mc][:], rb[:],
                                                op=mybir.AluOpType.mult)
                        nc.vector.tensor_tensor(
                            o_sb[:, mc, 512 * ic2:512 * ic2 + 512], t1[:],
                            convb_sb[km][:, mc, 512 * ic2:512 * ic2 + 512],
                            op=mybir.AluOpType.add)

                    # up-projection + bias + input residual for this
                    # query-half (n2 == ic2), so the tail never waits for
                    # the other half
                    n2 = ic2
                    for oc in range(4):
                        psu = pp.tile([128, 512], F32, tag="ps")
                        for kc in range(2):
                            nc.tensor.matmul(
                                psu[:],
                                sb[f"upw_{km}"][:, kc, 128 * oc:128 * oc + 128],
                                o_sb[:, kc, 512 * n2:512 * n2 + 512],
                                start=(kc == 0), stop=(kc == 1))
                        tb = epool.tile([128, 512], F16, tag="tb")
                        nc.vector.tensor_scalar_add(
                            tb[:], psu[:], sb[f"upb_{km}"][:, oc:oc + 1])
                        ob = epool.tile([128, 512], F32, tag="ob")
                        nc.vector.tensor_tensor(
                            ob[:], tb[:],
                            sb[f"xs_{km}"][:, 1, oc,
                                           1 + 8 * n2: 9 + 8 * n2, :],
                            op=mybir.AluOpType.add)
                        nc.sync.dma_start(
                            OUT[km][128 * oc:128 * oc + 128,
                                    512 * n2:512 * n2 + 512], ob[:])


    nc.compile()
    return nc


@functools.lru_cache(maxsize=1)
def _program():
    return _build()


def _prep_shared(inputs):
    W_ = {}
    for m in MODS:
        cw = np.asarray(inputs[f"conv_w_{m}"], np.float32)       # [CD,CIN,3,3]
        W_[f"cw_{m}"] = np.ascontiguousarray(
            cw.transpose(1, 2, 3, 0).reshape(4, 128, 3, 3, CD)
              .transpose(2, 3, 0, 1, 4).reshape(9, 4, 128, CD)).astype(np.float16)
        g = np.asarray(inputs[f"bn_g_{m}"], np.float64)
        bb = np.asarray(inputs[f"bn_b_{m}"], np.float64)
        mu = np.asarray(inputs[f"bn_m_{m}"], np.float64)
        v = np.asarray(inputs[f"bn_v_{m}"], np.float64)
        cb = np.asarray(inputs[f"conv_b_{m}"], np.float64)
        scale = (g / np.sqrt(v + 1e-5))
        shift = bb - mu * scale + cb * scale     # fold conv bias into BN shift
        W_[f"bna_{m}"] = np.ascontiguousarray(
            scale.astype(np.float32).reshape(2, 128).T)
        W_[f"bnb_{m}"] = np.ascontiguousarray(
            shift.astype(np.float32).reshape(2, 128).T)
        W_[f"alpha_{m}"] = np.full((128, 1),
                                   np.float32(inputs[f"prelu_{m}"]), np.float32)
        W_[f"gamma_{m}"] = np.asarray(inputs[f"gamma_{m}"],
                                      np.float32).reshape(1, 1)
        qk = np.concatenate([np.asarray(inputs[f"q_w_{m}"], np.float32),
                             np.asarray(inputs[f"k_w_{m}"], np.float32)], 0)
        W_[f"qkw_{m}"] = np.ascontiguousarray(
            qk.T.reshape(2, 128, 64)).astype(np.float16)
        W_[f"qkb_{m}"] = np.concatenate(
            [np.asarray(inputs[f"q_b_{m}"], np.float32),
             np.asarray(inputs[f"k_b_{m}"], np.float32)], 0).reshape(64, 1)
        W_[f"vw_{m}"] = np.ascontiguousarray(
            np.asarray(inputs[f"v_w_{m}"], np.float32).T
            .reshape(2, 128, CD)).astype(np.float16)
        W_[f"upw_{m}"] = np.ascontiguousarray(
            np.asarray(inputs[f"up_w_{m}"], np.float32).T
            .reshape(2, 128, CIN)).astype(np.float16)
        W_[f"upb_{m}"] = np.ascontiguousarray(
            np.asarray(inputs[f"up_b_{m}"], np.float32).reshape(4, 128).T)
        gvb = (np.float32(inputs[f"gamma_{m}"])
               * np.asarray(inputs[f"v_b_{m}"], np.float32))
        W_[f"gvb_{m}"] = np.ascontiguousarray(gvb.reshape(2, 128).T)
    W_["negI"] = -np.eye(128, dtype=np.float32)
    return W_


def _slab(x_b, s):
    xp = np.zeros((CIN, HR, WP), np.float32)
    r0 = SLAB_ROWS * s - 1
    lo, hi = max(r0, 0), min(r0 + HR, H)
    xp[:, lo - r0:hi - r0, 1:1 + W] = x_b[:, lo:hi, :]
    # three pre-shifted copies so the conv matmul moving operand is contiguous
    x3 = np.stack([xp[:, :, dx:dx + W] for dx in range(3)], 0)  # [3,CIN,HR,64]
    return np.ascontiguousarray(
        x3.reshape(3, 4, 128, HR, W).transpose(2, 0, 1, 3, 4)).astype(np.float16)


def kernel(**inputs):
    nc = _program()
    W_ = _prep_shared(inputs)
    xin = {m: np.asarray(inputs[f"input_{m}"], np.float32) for m in MODS}
    in_maps = []
    for cid in range(N_CORES):
        b, s = cid // 4, cid % 4
        im = dict(W_)
        for m in MODS:
            im[f"xs_{m}"] = _slab(xin[m][b], s)
        in_maps.append(im)
    res = run_bass_kernel_spmd(nc, in_maps, core_ids=list(range(N_CORES)))
    out = {m: np.zeros((B, CIN, H, W), np.float32) for m in MODS}
    for cid in range(N_CORES):
        b, s = cid // 4, cid % 4
        for m in MODS:
            out[m][b, :, SLAB_ROWS * s:SLAB_ROWS * (s + 1), :] = (
                res.results[cid][f"out_{m}"].reshape(CIN, SLAB_ROWS, W))
    return (out["rgb"], out["dsm"])
